# revision 9
# baseline (speedup 1.0000x reference)
"""Multi-head causal attention (B=2, S=2048, D=1024, H=16) on 8 TRN2 cores.

Sharding: core = (batch b = core//4, head-group g = core%4). Each core
computes 4 heads of one batch end-to-end (QKV projections for its head
slice, causal attention, its partial contribution to the output
projection). Host sums the 4 partial outputs per batch and adds the bias.

Device algorithm (per core), all matmuls in bf16 with f32 PSUM accum:
  inputs x.T ride 12 big (1 MB) zero-dependency DMAs issued up front
  (q/k on the sync HWDGE ring, v on the vector ring, weights on the
  scalar ring) into persistent SBUF tiles, so compute is never blocked
  on a late descriptor.
  qT/kT [dloc=256, S] = Wslice @ x.T ; V [S, dloc] per s-tile.
  The V-projection matmuls (N=256, stationary = x slices) are emitted
  alternating with the N=512 q/k matmuls so every LDWEIGHTS hides under
  the previous matmul's stream (a pure N=256 run is LDWEIGHTS-bound).
  attention per (q-chunk of 512, head pair):
    sT[k,q] both heads -> one 2-bank PSUM tile (row-tiled pair, K=64)
    attnT = exp(sT * 1/8) via one strided ScalarE op per k-tile pair;
    causal: k-tiles above the diagonal skipped, diagonal tiles use a
    q-subrange; only the 128x128 block ON the diagonal needs the 0/1
    mask multiply.
    per head: AT_aug [65, q] = sum_k V_aug.T @ attnT  -> PSUM, where
    V_aug column 0 is ones so row 0 of the PSUM is the softmax
    denominator l at partition 0 (reciprocal_approx_fast reads it
    directly); AT = AT * bcast(1/l) via GpSimd partition broadcast.
  out_partial [S, 1024] = AT matmul with the Wo slice, staged to bf16
  (host sums partials in f32, adds bias).
Q/K/V projections for upcoming chunks and the Wo projection for
finished chunks are emitted interleaved with attention so the PE always
has dense independent work while ScalarE exp catches up.

The device kernel assumes the causal (lower-triangular) mask the
reference constructs; kernel() verifies that and falls back to an exact
numpy implementation for any other mask.
"""

import numpy as np
import ml_dtypes

D_MODEL = 1024
NUM_HEADS = 16
HEAD_DIM = 64
B = 2
S = 2048
N_CORES = 8
GROUPS = 4                 # head-groups (cores per batch)
HPC = NUM_HEADS // GROUPS  # 4 heads per core
DLOC = HPC * HEAD_DIM      # 256 local projection dims
P = 128
SCH = 512                  # q/s chunk
NCH = S // SCH             # 4
KT = S // P                # 16 k-tiles
IT = D_MODEL // P          # 8 contraction tiles
MB = DLOC // P             # 2 m-blocks

_CACHE = {}


def _build():
    import concourse.bass as bass
    import concourse.tile as tile
    from concourse import bacc, mybir

    F32 = mybir.dt.float32
    BF16 = mybir.dt.bfloat16

    nc = bacc.Bacc("TRN2", target_bir_lowering=False, debug=False,
                   num_devices=N_CORES)

    # x inputs host-tiled chunk-major: [NCH, IT, 128, 512]
    xq = nc.dram_tensor("xq_t", [NCH, IT, P, SCH], BF16, kind="ExternalInput")
    xk = nc.dram_tensor("xk_t", [NCH, IT, P, SCH], BF16, kind="ExternalInput")
    xv = nc.dram_tensor("xv_t", [NCH, IT, P, SCH], BF16, kind="ExternalInput")
    wq = nc.dram_tensor("wq_t", [D_MODEL, DLOC], BF16, kind="ExternalInput")
    wk = nc.dram_tensor("wk_t", [D_MODEL, DLOC], BF16, kind="ExternalInput")
    wv = nc.dram_tensor("wv_t", [D_MODEL, DLOC], BF16, kind="ExternalInput")
    wo = nc.dram_tensor("wo_t", [DLOC, D_MODEL], BF16, kind="ExternalInput")
    mk = nc.dram_tensor("mask", [P, P], BF16, kind="ExternalInput")
    outp = nc.dram_tensor("outp", [S, D_MODEL], BF16, kind="ExternalOutput")

    Exp = mybir.ActivationFunctionType.Exp

    with tile.TileContext(nc) as tc:
        with (
            tc.tile_pool(name="const", bufs=1) as constp,
            tc.tile_pool(name="persist", bufs=1) as pers,
            tc.tile_pool(name="attn", bufs=19) as attnp,
            tc.tile_pool(name="small", bufs=3) as small,
            tc.tile_pool(name="ostage", bufs=6) as ostage,
            tc.tile_pool(name="psA", bufs=2, space="PSUM") as psA,
            tc.tile_pool(name="psS", bufs=2, space="PSUM") as psS,
            tc.tile_pool(name="psO", bufs=2, space="PSUM") as psO,
        ):
            # ---- constants / persistent tensors ----
            wq_sb = constp.tile([P, IT, DLOC], BF16)
            wk_sb = constp.tile([P, IT, DLOC], BF16)
            wv_sb = constp.tile([P, IT, DLOC], BF16)
            wo_sb = constp.tile([P, MB, D_MODEL], BF16)
            mk_sb = constp.tile([P, P], BF16)

            xq_sb = pers.tile([P, NCH, IT, SCH], BF16)
            xk_sb = pers.tile([P, NCH, IT, SCH], BF16)
            xv_sb = pers.tile([P, NCH, IT, SCH], BF16)

            qT_sb = pers.tile([P, MB, S], BF16)
            kT_sb = pers.tile([P, MB, S], BF16)
            v_sb = pers.tile([P, KT, HPC, HEAD_DIM + 1], BF16)
            atn_sb = pers.tile([P, MB, S], BF16)

            # weights on the scalar ring, in first-use order
            nc.scalar.dma_start(wq_sb[:], wq[:].rearrange("(r p) d -> p r d", p=P))
            nc.scalar.dma_start(wv_sb[:], wv[:].rearrange("(r p) d -> p r d", p=P))
            nc.scalar.dma_start(wk_sb[:], wk[:].rearrange("(r p) d -> p r d", p=P))
            nc.scalar.dma_start(mk_sb[:], mk[:])
            nc.scalar.dma_start(wo_sb[:], wo[:].rearrange("(m p) o -> p m o", p=P))

            # q/k chunk slabs on the sync ring, v on the vector ring;
            # no dependencies — these all issue immediately.
            for c in range(NCH):
                nc.sync.dma_start(xq_sb[:, c], xq[c].rearrange("r p s -> p r s"))
                nc.sync.dma_start(xk_sb[:, c], xk[c].rearrange("r p s -> p r s"))
            for c in range(NCH):
                nc.gpsimd.dma_start(xv_sb[:, c], xv[c].rearrange("r p s -> p r s"))

            nc.vector.memset(v_sb[:, :, :, HEAD_DIM:HEAD_DIM + 1], 1.0)

            def proj_sweep(c, part, m, j):
                # one N=512 projection stream (q or k, m-block m) for chunk
                # c interleaved with the N=256 V matmuls for s-tile 4c+j
                x_sb, w_sb, dst = ((xq_sb, wq_sb, qT_sb) if part == 0 else
                                   (xk_sb, wk_sb, kT_sb))
                ps = psA.tile([P, SCH], F32, tag="psA", name="psqk")
                ps_v = psA.tile([P, DLOC], F32, tag="psA", name="psv")
                for r in range(IT):
                    nc.tensor.matmul(
                        ps[:], w_sb[:, r, m * P:(m + 1) * P],
                        x_sb[:, c, r, :],
                        start=(r == 0), stop=(r == IT - 1))
                    nc.tensor.matmul(
                        ps_v[:], xv_sb[:, c, r, j * P:(j + 1) * P],
                        wv_sb[:, r, :],
                        start=(r == 0), stop=(r == IT - 1))
                nc.vector.tensor_copy(dst[:, m, c * SCH:(c + 1) * SCH], ps[:])
                nc.vector.tensor_copy(
                    v_sb[:, 4 * c + j, :, 0:HEAD_DIM],
                    ps_v[:].rearrange("p (h d) -> p h d", h=HPC))

            def qkv_proj(c, half=None):
                # half 0: q sweeps (+ v s-tiles 0,1); half 1: k (+ v 2,3)
                halves = (0, 1) if half is None else (half,)
                for hf in halves:
                    for m in range(MB):
                        proj_sweep(c, hf, m, 2 * hf + m)

            def wo_proj(c, ts=(0, 1, 2, 3)):
                # output projection for s-tiles 4c+ts, bf16 partials out
                for t in [4 * c + i for i in ts]:
                    for oc in range(D_MODEL // SCH):
                        ps_o = psA.tile([P, SCH], F32, tag="psA", name="pso")
                        for m in range(MB):
                            nc.tensor.matmul(
                                ps_o[:], atn_sb[:, m, t * P:(t + 1) * P],
                                wo_sb[:, m, oc * SCH:(oc + 1) * SCH],
                                start=(m == 0), stop=(m == MB - 1))
                        ot = ostage.tile([P, SCH], BF16, tag="ot")
                        nc.vector.tensor_copy(ot[:], ps_o[:])
                        nc.gpsimd.dma_start(
                            outp[t * P:(t + 1) * P, oc * SCH:(oc + 1) * SCH], ot[:])

            def norm_head(h, c, ps_at):
                # AT[0:64] *= broadcast(1/l);  l = ps_at row 64.
                # approx_fast mishandles partition-offset inputs: stage the
                # l row to partition 0 first.
                m, po = h // 2, (h % 2) * HEAD_DIM
                lrow = small.tile([1, SCH], F32, tag="lrow")
                nc.vector.tensor_copy(lrow[:], ps_at[HEAD_DIM:HEAD_DIM + 1, :])
                linv = small.tile([1, SCH], F32, tag="linv")
                nc.vector.reciprocal_approx_fast(out=linv[:], in_=lrow[:])
                lbc = small.tile([HEAD_DIM, SCH], F32, tag="lbc")
                nc.gpsimd.partition_broadcast(lbc[:], linv[:])
                nc.vector.tensor_mul(
                    atn_sb[po:po + HEAD_DIM, m, c * SCH:(c + 1) * SCH],
                    ps_at[0:HEAD_DIM, :], lbc[:])

            qkv_proj(0)

            # ---- attention: chunk-major, head pairs, PE filler interleaved ----
            for c in range(NCH):
                nkt = 4 * (c + 1)  # causal: k-tiles 0..nkt-1

                def qoff(kt):
                    # diagonal k-tile j only needs q in [128j, 512)
                    return max(kt - 4 * c, 0) * P

                for hp in range(HPC // 2):
                    atts = []
                    for kt in range(nkt):
                        qo = qoff(kt)
                        ps_s = psS.tile([P, 2, SCH], F32, tag="psS")
                        for hh in range(2):
                            h = 2 * hp + hh
                            m, po = h // 2, (h % 2) * HEAD_DIM
                            nc.tensor.matmul(
                                ps_s[:, hh, qo:],
                                kT_sb[po:po + HEAD_DIM, m, kt * P:(kt + 1) * P],
                                qT_sb[po:po + HEAD_DIM, m,
                                      c * SCH + qo:(c + 1) * SCH],
                                start=True, stop=True)
                        att = attnp.tile([P, 2, SCH], BF16, tag="attn")
                        nc.scalar.activation(att[:, :, qo:],
                                             ps_s[:, :, qo:], Exp, scale=0.125)
                        if kt - 4 * c >= 0:
                            # the 128x128 block on the diagonal is the only
                            # one needing the triangular mask
                            for hh in range(2):
                                nc.vector.tensor_mul(
                                    att[:, hh, qo:qo + P], att[:, hh, qo:qo + P],
                                    mk_sb[:])
                        atts.append(att)
                    for hh in range(2):
                        h = 2 * hp + hh
                        ps_at = psO.tile([HEAD_DIM + 1, SCH], F32, tag="psO",
                                         name="ps_at")
                        for kt in range(nkt):
                            qo = qoff(kt)
                            nc.tensor.matmul(
                                ps_at[:, qo:], v_sb[:, kt, h, :],
                                atts[kt][:, hh, qo:],
                                start=(kt == 0), stop=(kt == nkt - 1))
                        norm_head(h, c, ps_at)

                    # PE filler between head pairs / chunks: projections for
                    # upcoming chunks + output projection for finished ones.
                    if hp == 0:
                        if c + 1 < NCH:
                            qkv_proj(c + 1, half=0)
                    else:
                        if c + 1 < NCH:
                            qkv_proj(c + 1, half=1)
                        if c >= 1:
                            # hold back half of the last filler so the PE has
                            # warm work during the final norm chains
                            wo_proj(c - 1, ts=(0, 1) if c == NCH - 1 else
                                    (0, 1, 2, 3))
            wo_proj(NCH - 2, ts=(2, 3))
            wo_proj(NCH - 1)

    nc.compile()
    return nc


def _get_nc():
    if "nc" not in _CACHE:
        _CACHE["nc"] = _build()
    return _CACHE["nc"]


def _mask_const():
    # triangular 128x128: mask[k, t] = 1.0 iff t >= k
    t = np.arange(P)[None, :]
    k = np.arange(P)[:, None]
    return (t >= k).astype(ml_dtypes.bfloat16)


def _tile_xt(x_t):
    # [D_MODEL, S] -> [NCH, IT, 128, 512] chunk-major contiguous tiles
    return np.ascontiguousarray(
        x_t.reshape(IT, P, NCH, SCH).transpose(2, 0, 1, 3))


def _kernel_numpy(query, key, value, mask, Wq, Wk, Wv, Wo, bo):
    # exact f32 fallback for non-causal masks
    q = (query @ Wq.T).reshape(B, S, NUM_HEADS, HEAD_DIM).transpose(0, 2, 1, 3)
    k = (key @ Wk.T).reshape(B, S, NUM_HEADS, HEAD_DIM).transpose(0, 2, 1, 3)
    v = (value @ Wv.T).reshape(B, S, NUM_HEADS, HEAD_DIM).transpose(0, 2, 1, 3)
    s = np.einsum("bhqd,bhkd->bhqk", q, k) / np.sqrt(np.float32(HEAD_DIM))
    s = np.where(np.asarray(mask), s, -np.inf)
    s = s - s.max(axis=-1, keepdims=True)
    e = np.exp(s)
    a = e / e.sum(axis=-1, keepdims=True)
    o = np.einsum("bhqk,bhkd->bhqd", a, v).transpose(0, 2, 1, 3)
    return (o.reshape(B, S, D_MODEL) @ Wo.T + bo).astype(np.float32)


def kernel(query, key, value, mask, Wq, Wk, Wv, Wo, bo):
    from concourse.bass_utils import run_bass_kernel_spmd

    m = np.asarray(mask).astype(bool)
    expect = np.tril(np.ones((S, S), dtype=bool))
    if m.size != S * S or not np.array_equal(m.reshape(S, S), expect):
        args = [np.asarray(a, np.float32) for a in
                (query, key, value)] + [mask] + [
                np.asarray(a, np.float32) for a in (Wq, Wk, Wv, Wo, bo)]
        return _kernel_numpy(*args)

    nc = _get_nc()
    bf = ml_dtypes.bfloat16

    xq_t = [_tile_xt(np.asarray(query)[b].T.astype(bf)) for b in range(B)]
    xk_t = [_tile_xt(np.asarray(key)[b].T.astype(bf)) for b in range(B)]
    xv_t = [_tile_xt(np.asarray(value)[b].T.astype(bf)) for b in range(B)]
    WqT = np.ascontiguousarray(np.asarray(Wq).T).astype(bf)  # [D, D] cols = out dim
    WkT = np.ascontiguousarray(np.asarray(Wk).T).astype(bf)
    WvT = np.ascontiguousarray(np.asarray(Wv).T).astype(bf)
    WoT = np.ascontiguousarray(np.asarray(Wo).T).astype(bf)
    mk = _mask_const()

    in_maps = []
    for core in range(N_CORES):
        b, g = core // GROUPS, core % GROUPS
        hsl = slice(g * DLOC, (g + 1) * DLOC)
        in_maps.append({
            "xq_t": xq_t[b], "xk_t": xk_t[b], "xv_t": xv_t[b],
            "wq_t": np.ascontiguousarray(WqT[:, hsl]),
            "wk_t": np.ascontiguousarray(WkT[:, hsl]),
            "wv_t": np.ascontiguousarray(WvT[:, hsl]),
            "wo_t": np.ascontiguousarray(WoT[hsl, :]),
            "mask": mk,
        })

    res = run_bass_kernel_spmd(nc, in_maps, core_ids=list(range(N_CORES)))
    _CACHE["last_result"] = res

    out = np.zeros((B, S, D_MODEL), np.float32)
    for core in range(N_CORES):
        out[core // GROUPS] += np.asarray(res.results[core]["outp"],
                                          dtype=np.float32)
    out += np.asarray(bo, np.float32)[None, None, :]
    return out


# revision 15
# speedup vs baseline: 1.0501x; 1.0501x over previous
"""Multi-head causal attention (B=2, S=2048, D=1024, H=16) on 8 TRN2 cores.

Sharding: core = (batch b = core//4, head-group g = core%4). Each core
computes 4 heads of one batch end-to-end (QKV projections for its head
slice, causal attention, its partial contribution to the output
projection). Host sums the 4 partial outputs per batch and adds the bias.

Device algorithm (per core), all matmuls in bf16 with f32 PSUM accum:
  inputs x.T ride 12 big (1 MB) zero-dependency DMAs issued up front
  (q/k on the sync HWDGE ring, v on the vector ring, weights on the
  scalar ring) into persistent SBUF tiles, so compute is never blocked
  on a late descriptor.
  qT/kT [dloc=256, S] = Wslice @ x.T ; V [S, dloc] per s-tile.
  The V-projection matmuls (N=256, stationary = x slices) are emitted
  alternating with the N=512 q/k matmuls so every LDWEIGHTS hides under
  the previous matmul's stream (a pure N=256 run is LDWEIGHTS-bound).
  attention per (q-chunk of 512, head pair):
    sT[k,q] both heads -> one 2-bank PSUM tile (row-tiled pair, K=64)
    attnT = exp(sT * 1/8) via one strided ScalarE op per k-tile pair;
    causal: k-tiles above the diagonal skipped, diagonal tiles use a
    q-subrange; only the 128x128 block ON the diagonal needs the 0/1
    mask multiply.
    per head: AT_aug [65, q] = sum_k V_aug.T @ attnT  -> PSUM, where
    V_aug column 0 is ones so row 0 of the PSUM is the softmax
    denominator l at partition 0 (reciprocal_approx_fast reads it
    directly); AT = AT * bcast(1/l) via GpSimd partition broadcast.
  out_partial [S, 1024] = AT matmul with the Wo slice, staged to bf16
  (host sums partials in f32, adds bias).
Q/K/V projections for upcoming chunks and the Wo projection for
finished chunks are emitted interleaved with attention so the PE always
has dense independent work while ScalarE exp catches up.

The device kernel assumes the causal (lower-triangular) mask the
reference constructs; kernel() verifies that and falls back to an exact
numpy implementation for any other mask.
"""

import numpy as np
import ml_dtypes

D_MODEL = 1024
NUM_HEADS = 16
HEAD_DIM = 64
B = 2
S = 2048
N_CORES = 8
GROUPS = 4                 # head-groups (cores per batch)
HPC = NUM_HEADS // GROUPS  # 4 heads per core
DLOC = HPC * HEAD_DIM      # 256 local projection dims
P = 128
SCH = 512                  # q/s chunk
NCH = S // SCH             # 4
KT = S // P                # 16 k-tiles
IT = D_MODEL // P          # 8 contraction tiles
MB = DLOC // P             # 2 m-blocks

_CACHE = {}


def _build():
    import concourse.bass as bass
    import concourse.tile as tile
    from concourse import bacc, mybir

    F32 = mybir.dt.float32
    BF16 = mybir.dt.bfloat16

    nc = bacc.Bacc("TRN2", target_bir_lowering=False, debug=False,
                   num_devices=N_CORES)

    # x inputs and weights host-prearranged into the per-partition SBUF
    # image so every DMA is 128 rows of >=4KB contiguous bytes (descriptor
    # generation cost scales with descriptor count, so 1KB-row scatters
    # are ~8x more expensive to issue).
    xq = nc.dram_tensor("xq_t", [P, NCH, IT, SCH], BF16, kind="ExternalInput")
    xk = nc.dram_tensor("xk_t", [P, NCH, IT, SCH], BF16, kind="ExternalInput")
    xv = nc.dram_tensor("xv_t", [P, NCH, IT, SCH], BF16, kind="ExternalInput")
    wq = nc.dram_tensor("wq_t", [P, IT, DLOC], BF16, kind="ExternalInput")
    wk = nc.dram_tensor("wk_t", [P, IT, DLOC], BF16, kind="ExternalInput")
    wv = nc.dram_tensor("wv_t", [P, IT, DLOC], BF16, kind="ExternalInput")
    wo = nc.dram_tensor("wo_t", [P, MB, D_MODEL], BF16, kind="ExternalInput")
    mk = nc.dram_tensor("mask", [P, P], BF16, kind="ExternalInput")
    outp = nc.dram_tensor("outp", [S, D_MODEL], BF16, kind="ExternalOutput")

    Exp = mybir.ActivationFunctionType.Exp

    with tile.TileContext(nc) as tc:
        with (
            tc.tile_pool(name="const", bufs=1) as constp,
            tc.tile_pool(name="persist", bufs=1) as pers,
            tc.tile_pool(name="attn", bufs=19) as attnp,
            tc.tile_pool(name="small", bufs=3) as small,
            tc.tile_pool(name="ostage", bufs=3) as ostage,
            tc.tile_pool(name="psA", bufs=2, space="PSUM") as psA,
            tc.tile_pool(name="psS", bufs=2, space="PSUM") as psS,
            tc.tile_pool(name="psO", bufs=2, space="PSUM") as psO,
        ):
            # ---- constants / persistent tensors ----
            wq_sb = constp.tile([P, IT, DLOC], BF16)
            wk_sb = constp.tile([P, IT, DLOC], BF16)
            wv_sb = constp.tile([P, IT, DLOC], BF16)
            wo_sb = constp.tile([P, MB, D_MODEL], BF16)
            mk_sb = constp.tile([P, P], BF16)

            xq_sb = pers.tile([P, NCH, IT, SCH], BF16)
            xk_sb = pers.tile([P, NCH, IT, SCH], BF16)
            xv_sb = pers.tile([P, NCH, IT, SCH], BF16)

            qT_sb = pers.tile([P, MB, S], BF16)
            kT_sb = pers.tile([P, MB, S], BF16)
            v_sb = pers.tile([P, KT, HPC, HEAD_DIM + 1], BF16)
            atn_sb = pers.tile([P, MB, S], BF16)

            # weights on the scalar ring, in first-use order
            nc.scalar.dma_start(wq_sb[:], wq[:])
            nc.scalar.dma_start(wv_sb[:], wv[:])
            nc.scalar.dma_start(wk_sb[:], wk[:])
            nc.scalar.dma_start(mk_sb[:], mk[:])
            nc.scalar.dma_start(wo_sb[:], wo[:])

            # q/k chunk slabs on the sync ring, v on the gpsimd ring;
            # no dependencies — these all issue immediately.
            for c in range(NCH):
                nc.sync.dma_start(xq_sb[:, c], xq[:, c])
                nc.sync.dma_start(xk_sb[:, c], xk[:, c])
            for c in range(NCH):
                nc.gpsimd.dma_start(xv_sb[:, c], xv[:, c])

            nc.vector.memset(v_sb[:, :, :, HEAD_DIM:HEAD_DIM + 1], 1.0)

            def proj_sweep(c, part, m, j):
                # one N=512 projection stream (q or k, m-block m) for chunk
                # c interleaved with the N=256 V matmuls for s-tile 4c+j
                x_sb, w_sb, dst = ((xq_sb, wq_sb, qT_sb) if part == 0 else
                                   (xk_sb, wk_sb, kT_sb))
                ps = psA.tile([P, SCH], F32, tag="psA", name="psqk")
                ps_v = psA.tile([P, DLOC], F32, tag="psA", name="psv")
                for r in range(IT):
                    nc.tensor.matmul(
                        ps[:], w_sb[:, r, m * P:(m + 1) * P],
                        x_sb[:, c, r, :],
                        start=(r == 0), stop=(r == IT - 1))
                    nc.tensor.matmul(
                        ps_v[:], xv_sb[:, c, r, j * P:(j + 1) * P],
                        wv_sb[:, r, :],
                        start=(r == 0), stop=(r == IT - 1))
                nc.vector.tensor_copy(dst[:, m, c * SCH:(c + 1) * SCH], ps[:])
                nc.vector.tensor_copy(
                    v_sb[:, 4 * c + j, :, 0:HEAD_DIM],
                    ps_v[:].rearrange("p (h d) -> p h d", h=HPC))

            def qkv_proj(c, half=None):
                # half 0: q sweeps (+ v s-tiles 0,1); half 1: k (+ v 2,3)
                halves = (0, 1) if half is None else (half,)
                for hf in halves:
                    for m in range(MB):
                        proj_sweep(c, hf, m, 2 * hf + m)

            def wo_proj(c, ts=(0, 1, 2, 3)):
                # output projection for s-tiles 4c+ts; both oc halves are
                # staged into one bf16 tile so the store is 128 full 2KB rows
                for t in [4 * c + i for i in ts]:
                    ot = ostage.tile([P, D_MODEL], BF16, tag="ot")
                    for oc in range(D_MODEL // SCH):
                        ps_o = psA.tile([P, SCH], F32, tag="psA", name="pso")
                        for m in range(MB):
                            nc.tensor.matmul(
                                ps_o[:], atn_sb[:, m, t * P:(t + 1) * P],
                                wo_sb[:, m, oc * SCH:(oc + 1) * SCH],
                                start=(m == 0), stop=(m == MB - 1))
                        nc.vector.tensor_copy(
                            ot[:, oc * SCH:(oc + 1) * SCH], ps_o[:])
                    nc.gpsimd.dma_start(outp[t * P:(t + 1) * P, :], ot[:])

            def norm_head(h, c, ps_at):
                # AT[0:64] *= broadcast(1/l);  l = ps_at row 64.
                # approx_fast mishandles partition-offset inputs: stage the
                # l row to partition 0 first.
                m, po = h // 2, (h % 2) * HEAD_DIM
                lrow = small.tile([1, SCH], F32, tag="lrow")
                nc.vector.tensor_copy(lrow[:], ps_at[HEAD_DIM:HEAD_DIM + 1, :])
                linv = small.tile([1, SCH], F32, tag="linv")
                nc.vector.reciprocal_approx_fast(out=linv[:], in_=lrow[:])
                lbc = small.tile([HEAD_DIM, SCH], F32, tag="lbc")
                nc.gpsimd.partition_broadcast(lbc[:], linv[:])
                nc.vector.tensor_mul(
                    atn_sb[po:po + HEAD_DIM, m, c * SCH:(c + 1) * SCH],
                    ps_at[0:HEAD_DIM, :], lbc[:])

            qkv_proj(0)

            # ---- attention: chunk-major, head pairs, PE filler interleaved ----
            for c in range(NCH):
                nkt = 4 * (c + 1)  # causal: k-tiles 0..nkt-1

                def qoff(kt):
                    # diagonal k-tile j only needs q in [128j, 512)
                    return max(kt - 4 * c, 0) * P

                for hp in range(HPC // 2):
                    atts = []
                    for kt in range(nkt):
                        qo = qoff(kt)
                        ps_s = psS.tile([P, 2, SCH], F32, tag="psS")
                        for hh in range(2):
                            h = 2 * hp + hh
                            m, po = h // 2, (h % 2) * HEAD_DIM
                            nc.tensor.matmul(
                                ps_s[:, hh, qo:],
                                kT_sb[po:po + HEAD_DIM, m, kt * P:(kt + 1) * P],
                                qT_sb[po:po + HEAD_DIM, m,
                                      c * SCH + qo:(c + 1) * SCH],
                                start=True, stop=True)
                        att = attnp.tile([P, 2, SCH], BF16, tag="attn")
                        nc.scalar.activation(att[:, :, qo:],
                                             ps_s[:, :, qo:], Exp, scale=0.125)
                        if kt - 4 * c >= 0:
                            # the 128x128 block on the diagonal is the only
                            # one needing the triangular mask
                            for hh in range(2):
                                nc.vector.tensor_mul(
                                    att[:, hh, qo:qo + P], att[:, hh, qo:qo + P],
                                    mk_sb[:])
                        atts.append(att)
                    for hh in range(2):
                        h = 2 * hp + hh
                        ps_at = psO.tile([HEAD_DIM + 1, SCH], F32, tag="psO",
                                         name="ps_at")
                        for kt in range(nkt):
                            qo = qoff(kt)
                            nc.tensor.matmul(
                                ps_at[:, qo:], v_sb[:, kt, h, :],
                                atts[kt][:, hh, qo:],
                                start=(kt == 0), stop=(kt == nkt - 1))
                        norm_head(h, c, ps_at)

                    # PE filler between head pairs / chunks: projections for
                    # upcoming chunks + output projection for finished ones.
                    if hp == 0:
                        if c + 1 < NCH:
                            qkv_proj(c + 1, half=0)
                    else:
                        if c + 1 < NCH:
                            qkv_proj(c + 1, half=1)
                        if c >= 1:
                            # hold back half of the last filler so the PE has
                            # warm work during the final norm chains
                            wo_proj(c - 1, ts=(0, 1) if c == NCH - 1 else
                                    (0, 1, 2, 3))
            wo_proj(NCH - 2, ts=(2, 3))
            wo_proj(NCH - 1)

    nc.compile()
    return nc


def _get_nc():
    if "nc" not in _CACHE:
        _CACHE["nc"] = _build()
    return _CACHE["nc"]


def _mask_const():
    # triangular 128x128: mask[k, t] = 1.0 iff t >= k
    t = np.arange(P)[None, :]
    k = np.arange(P)[:, None]
    return (t >= k).astype(ml_dtypes.bfloat16)


def _tile_xt(x_t):
    # [D_MODEL, S] -> [128, NCH, IT, 512]: the per-partition SBUF image
    return np.ascontiguousarray(
        x_t.reshape(IT, P, NCH, SCH).transpose(1, 2, 0, 3))


def _tile_w(w, blocks):
    # [(blocks*128), N] -> [128, blocks, N]: the per-partition SBUF image
    n = w.shape[1]
    return np.ascontiguousarray(w.reshape(blocks, P, n).transpose(1, 0, 2))


def _kernel_numpy(query, key, value, mask, Wq, Wk, Wv, Wo, bo):
    # exact f32 fallback for non-causal masks
    q = (query @ Wq.T).reshape(B, S, NUM_HEADS, HEAD_DIM).transpose(0, 2, 1, 3)
    k = (key @ Wk.T).reshape(B, S, NUM_HEADS, HEAD_DIM).transpose(0, 2, 1, 3)
    v = (value @ Wv.T).reshape(B, S, NUM_HEADS, HEAD_DIM).transpose(0, 2, 1, 3)
    s = np.einsum("bhqd,bhkd->bhqk", q, k) / np.sqrt(np.float32(HEAD_DIM))
    s = np.where(np.asarray(mask), s, -np.inf)
    s = s - s.max(axis=-1, keepdims=True)
    e = np.exp(s)
    a = e / e.sum(axis=-1, keepdims=True)
    o = np.einsum("bhqk,bhkd->bhqd", a, v).transpose(0, 2, 1, 3)
    return (o.reshape(B, S, D_MODEL) @ Wo.T + bo).astype(np.float32)


def kernel(query, key, value, mask, Wq, Wk, Wv, Wo, bo):
    from concourse.bass_utils import run_bass_kernel_spmd

    m = np.asarray(mask).astype(bool)
    expect = np.tril(np.ones((S, S), dtype=bool))
    if m.size != S * S or not np.array_equal(m.reshape(S, S), expect):
        args = [np.asarray(a, np.float32) for a in
                (query, key, value)] + [mask] + [
                np.asarray(a, np.float32) for a in (Wq, Wk, Wv, Wo, bo)]
        return _kernel_numpy(*args)

    nc = _get_nc()
    bf = ml_dtypes.bfloat16

    xq_t = [_tile_xt(np.asarray(query)[b].T.astype(bf)) for b in range(B)]
    xk_t = [_tile_xt(np.asarray(key)[b].T.astype(bf)) for b in range(B)]
    xv_t = [_tile_xt(np.asarray(value)[b].T.astype(bf)) for b in range(B)]
    WqT = np.ascontiguousarray(np.asarray(Wq).T).astype(bf)  # [D, D] cols = out dim
    WkT = np.ascontiguousarray(np.asarray(Wk).T).astype(bf)
    WvT = np.ascontiguousarray(np.asarray(Wv).T).astype(bf)
    WoT = np.ascontiguousarray(np.asarray(Wo).T).astype(bf)
    mk = _mask_const()

    in_maps = []
    for core in range(N_CORES):
        b, g = core // GROUPS, core % GROUPS
        hsl = slice(g * DLOC, (g + 1) * DLOC)
        in_maps.append({
            "xq_t": xq_t[b], "xk_t": xk_t[b], "xv_t": xv_t[b],
            "wq_t": _tile_w(WqT[:, hsl], IT),
            "wk_t": _tile_w(WkT[:, hsl], IT),
            "wv_t": _tile_w(WvT[:, hsl], IT),
            "wo_t": _tile_w(WoT[hsl, :], MB),
            "mask": mk,
        })

    res = run_bass_kernel_spmd(nc, in_maps, core_ids=list(range(N_CORES)))
    _CACHE["last_result"] = res

    out = np.zeros((B, S, D_MODEL), np.float32)
    for core in range(N_CORES):
        out[core // GROUPS] += np.asarray(res.results[core]["outp"],
                                          dtype=np.float32)
    out += np.asarray(bo, np.float32)[None, None, :]
    return out
